# revision 16
# baseline (speedup 1.0000x reference)
"""Llama MHA (B=2, S=2048, D=2048, H=16, causal, RoPE) on 8 trn2 cores.

Sharding: data-parallel over batch (2 groups of 4 cores) x tensor-parallel
over heads (4 heads per core). Each core computes, for its (batch, 4 heads):
  qT/kT = w^T x^T  (features on partitions, seq on free dim)
  RoPE on qT/kT (weights column-permuted on host so even/odd feature pairs
  land de-interleaved: rows 0:64 = even, 64:128 = odd; dot products are
  permutation-invariant so scores match the reference exactly)
  scoresT[k,q] blocks -> exp (no max subtraction needed: |score*scale| <~ 6)
  -> causal mask on diagonal blocks -> PV + ones-row denominator matmuls
  -> normalize -> out projection partial resT = wo^T attnT.
Host sums the 4 partials per batch and transposes back.

All matmuls in bf16 (fp32 PSUM accumulation); softmax/normalization in fp32.
"""

import numpy as np
import ml_dtypes

import concourse.bass as bass
import concourse.mybir as mybir
import concourse.tile as tile
from concourse import bacc
from concourse.bass_utils import run_bass_kernel_spmd

B, S, D, H = 2, 2048, 2048, 16
DH = D // H            # 128 head dim
HPC = 4                # heads per core
N_CORES = 8
FH = HPC * DH          # 512 features per core
P = 128
KT = D // P            # 16 k-tiles over D
SC = S // 512          # 4 seq chunks of 512
ST = S // P            # 16 seq blocks of 128
THETA = 10000.0
SCALE = 1.0 / np.sqrt(DH)

DT = mybir.dt.bfloat16
NPDT = ml_dtypes.bfloat16

_prog_cache = {}


def _build():
    if "nc" in _prog_cache:
        return _prog_cache["nc"]
    nc = bacc.Bacc(None, target_bir_lowering=False, debug=False)

    xT = nc.dram_tensor("xT", [D, S], DT, kind="ExternalInput")
    wq = nc.dram_tensor("wq", [D, FH], DT, kind="ExternalInput")
    wk = nc.dram_tensor("wk", [D, FH], DT, kind="ExternalInput")
    wv = nc.dram_tensor("wv", [D, FH], DT, kind="ExternalInput")
    wo = nc.dram_tensor("wo", [FH, D], DT, kind="ExternalInput")
    cc = nc.dram_tensor("cc", [P, S], mybir.dt.float32, kind="ExternalInput")
    ss = nc.dram_tensor("ss", [P, S], mybir.dt.float32, kind="ExternalInput")
    masks = nc.dram_tensor("masks", [P, 4, 512], DT, kind="ExternalInput")
    resT = nc.dram_tensor("resT", [D, S], mybir.dt.float32, kind="ExternalOutput")

    f32 = mybir.dt.float32

    with tile.TileContext(nc) as tc:
        with (
            tc.tile_pool(name="persist", bufs=1) as pp,
            tc.tile_pool(name="psA", bufs=4, space="PSUM") as psA,
            tc.tile_pool(name="psO", bufs=2, space="PSUM") as psO,
            tc.tile_pool(name="psD", bufs=2, space="PSUM") as psD,
        ):
            qT = pp.tile([P, HPC, S], DT)     # per head: rows=feat, free=seq
            kT = pp.tile([P, HPC, S], DT)
            vn = pp.tile([P, ST, FH], DT)     # v natural: [seq-block, feat]
            attnT = pp.tile([P, HPC, S], DT)  # normalized attention output^T
            cc_t = pp.tile([P, S], f32)
            ss_t = pp.tile([P, S], f32)
            mask_t = pp.tile([P, 4, 512], DT)
            ones_mat = pp.tile([P, P], DT)    # denominator stationary: the
                                              # [128,128] all-ones matrix makes
                                              # every PSUM row the key-sum, so
                                              # the broadcast is free

            nc.vector.memset(ones_mat, 1.0)
            wo_t = pp.tile([P, HPC, D], DT)

            # ---------------- Phase 1: projections + RoPE -----------------
            with (
                tc.tile_pool(name="wpool", bufs=1) as wp,
                tc.tile_pool(name="xpool", bufs=2) as xp,
                tc.tile_pool(name="ropetmp", bufs=4) as rp,
            ):
                wq_t = wp.tile([P, KT, FH], DT)
                wk_t = wp.tile([P, KT, FH], DT)
                wv_t = wp.tile([P, KT, FH], DT)
                # DMA issue order is the Sync-queue order: interleave the
                # first x chunk with wq so the first matmul chain starts as
                # early as possible; defer everything not needed immediately.
                xc0 = xp.tile([P, KT, 512], DT, tag="xc", name="xc0")
                for g in range(4):
                    gs = slice(g * 4, (g + 1) * 4)
                    nc.sync.dma_start(
                        out=wq_t[:, gs, :],
                        in_=wq.rearrange("(kt p) f -> p kt f", p=P)[:, gs, :])
                    nc.sync.dma_start(
                        out=xc0[:, gs, :],
                        in_=xT.rearrange("(kt p) s -> p kt s", p=P)[:, gs, 0:512])
                nc.sync.dma_start(out=cc_t[:, 0:512], in_=cc[:, 0:512])
                nc.sync.dma_start(out=ss_t[:, 0:512], in_=ss[:, 0:512])
                for g in range(4):
                    gs = slice(g * 4, (g + 1) * 4)
                    nc.sync.dma_start(
                        out=wk_t[:, gs, :],
                        in_=wk.rearrange("(kt p) f -> p kt f", p=P)[:, gs, :])
                nc.sync.dma_start(out=wv_t, in_=wv.rearrange("(kt p) f -> p kt f", p=P))
                nc.sync.dma_start(out=mask_t, in_=masks[:, :, :])
                nc.sync.dma_start(out=cc_t[:, 512:], in_=cc[:, 512:])
                nc.sync.dma_start(out=ss_t[:, 512:], in_=ss[:, 512:])

                for sc in range(SC):
                    if sc == 0:
                        xc = xc0
                    else:
                        xc = xp.tile([P, KT, 512], DT, tag="xc", name=f"xc{sc}")
                        for g in range(4):
                            gs = slice(g * 4, (g + 1) * 4)
                            nc.sync.dma_start(
                                out=xc[:, gs, :],
                                in_=xT.rearrange("(kt p) s -> p kt s", p=P)[
                                    :, gs, sc * 512:(sc + 1) * 512],
                            )
                    if sc == 1:
                        nc.sync.dma_start(
                            out=wo_t, in_=wo.rearrange("(ft p) d -> p ft d", p=P))
                    csl = slice(sc * 512, (sc + 1) * 512)
                    # q/k projections with RoPE fused into the PSUM drain
                    for h in range(HPC):
                        fsl = slice(h * DH, (h + 1) * DH)
                        for wt, dst in ((wq_t, qT), (wk_t, kT)):
                            pq = psA.tile([P, 512], f32, tag="ps", name=f"pq{sc}{h}")
                            for k in range(KT):
                                nc.tensor.matmul(
                                    pq, wt[:, k, fsl], xc[:, k, :],
                                    start=(k == 0), stop=(k == KT - 1),
                                )
                            # RoPE: dst = pq*cc + swap(pq)*(+/-ss)
                            # ss_t rows 0:64 = +sin (feeds bottom), rows
                            # 64:128 = -sin (feeds top); swap is done by
                            # writing each product into the opposite half
                            # so every DVE op has aligned base partitions.
                            ta = rp.tile([P, 512], f32, tag="ta")
                            tb = rp.tile([P, 512], f32, tag="tb")
                            nc.vector.tensor_mul(ta, pq, cc_t[:, csl])
                            nc.vector.tensor_mul(
                                tb[0:64, :], pq[64:128, :], ss_t[64:128, csl])
                            nc.vector.tensor_mul(
                                tb[64:128, :], pq[0:64, :], ss_t[0:64, csl])
                            nc.vector.tensor_add(dst[:, h, csl], ta, tb)
                    # v projection straight into natural layout
                    for st4 in range(4):
                        sb = sc * 4 + st4
                        pv = psA.tile([P, FH], f32, tag="ps", name=f"pv{sc}{st4}")
                        for k in range(KT):
                            nc.tensor.matmul(
                                pv, xc[:, k, st4 * P:(st4 + 1) * P], wv_t[:, k, :],
                                start=(k == 0), stop=(k == KT - 1),
                            )
                        nc.vector.tensor_copy(vn[:, sb, :], pv)

            # ---------------- Phase 2: attention ------------------------
            with (
                tc.tile_pool(name="ppool", bufs=4) as ptp,
                tc.tile_pool(name="npool", bufs=4) as np_,
            ):
                for qc in range(SC):
                    qsl = slice(qc * 512, (qc + 1) * 512)
                    for h in range(HPC):
                        fsl = slice(h * DH, (h + 1) * DH)
                        po = psO.tile([P, 512], f32, tag="po", name=f"po{h}{qc}")
                        pd = psD.tile([P, 512], f32, tag="pd", name=f"pd{h}{qc}")
                        nkb = 4 * qc + 4
                        prev_pt = None
                        for kb in range(nkb):
                            ps = psA.tile([P, 512], f32, tag="ps",
                                          name=f"ps{h}{qc}{kb}")
                            nc.tensor.matmul(
                                ps, kT[:, h, kb * P:(kb + 1) * P], qT[:, h, qsl],
                                start=True, stop=True,
                            )
                            pt = ptp.tile([P, 512], DT, tag="pt")
                            nc.scalar.activation(
                                pt, ps, mybir.ActivationFunctionType.Exp,
                                scale=float(SCALE),
                            )
                            if kb >= 4 * qc:
                                nc.vector.tensor_mul(
                                    pt, pt, mask_t[:, kb - 4 * qc, :])
                            nc.tensor.matmul(
                                po, vn[:, kb, fsl], pt,
                                start=(kb == 0), stop=(kb == nkb - 1),
                            )
                            if kb % 2 == 0:
                                prev_pt = pt
                            else:
                                # sum the pair on DVE; denominator matmul per
                                # pair halves the extra PE streams
                                psum2 = ptp.tile([P, 512], DT, tag="pt",
                                                 name=f"pp{h}{qc}{kb}")
                                nc.vector.tensor_add(psum2, prev_pt, pt)
                                nc.tensor.matmul(
                                    pd, ones_mat, psum2,
                                    start=(kb == 1), stop=(kb == nkb - 1),
                                )
                        bc = np_.tile([P, 512], f32, tag="bc")
                        nc.vector.reciprocal_approx_fast(out=bc, in_=pd)
                        nc.vector.tensor_mul(attnT[:, h, qsl], po, bc)

            # ---------------- Phase 3: output projection ----------------
            with (
                tc.tile_pool(name="rpool", bufs=4) as rop,
            ):
                for db in range(KT):
                    rt = rop.tile([P, S], f32, tag="rt")
                    for sc in range(SC):
                        csl = slice(sc * 512, (sc + 1) * 512)
                        pr = psA.tile([P, 512], f32, tag="ps", name=f"pr{sc}{db}")
                        for ft in range(HPC):
                            nc.tensor.matmul(
                                pr, wo_t[:, ft, db * P:(db + 1) * P],
                                attnT[:, ft, csl],
                                start=(ft == 0), stop=(ft == HPC - 1),
                            )
                        nc.vector.tensor_copy(rt[:, csl], pr)
                    nc.sync.dma_start(
                        out=resT[db * P:(db + 1) * P, :], in_=rt)

    nc.finalize()
    _prog_cache["nc"] = nc
    return nc


def _host_inputs(x, w_q, w_k, w_v, w_o):
    """Build the 8 per-core input maps."""
    # RoPE de-interleave permutation per head: evens then odds
    i = np.arange(DH)
    perm_head = np.concatenate([i[0::2], i[1::2]])  # within-head column order

    t = np.arange(S, dtype=np.float64)
    inv_freq = 1.0 / (THETA ** (np.arange(0, DH, 2, dtype=np.float64) / DH))
    ang = np.outer(t, inv_freq)          # [S, 64]
    cosT = np.cos(ang).T.astype(np.float32)   # [64, S]
    sinT = np.sin(ang).T.astype(np.float32)
    cc = np.vstack([cosT, cosT])         # [128, S]
    ss = np.vstack([sinT, -sinT])        # +sin feeds bottom half, -sin top

    # diagonal causal masks: mask[j][k, q] = 1 if 128*j + k <= q
    kk = np.arange(P)[:, None]
    qq = np.arange(512)[None, :]
    masks = np.stack(
        [(P * j + kk <= qq) for j in range(4)], axis=1
    ).astype(NPDT)                        # [128, 4, 512]

    in_maps = []
    for core in range(N_CORES):
        b = core // 4
        h0 = (core % 4) * HPC
        cols = np.concatenate(
            [h * DH + perm_head for h in range(h0, h0 + HPC)])   # rope-permuted
        vcols = np.arange(h0 * DH, (h0 + HPC) * DH)              # natural
        in_maps.append({
            "xT": np.ascontiguousarray(x[b].T).astype(NPDT),
            "wq": np.ascontiguousarray(w_q[:, cols]).astype(NPDT),
            "wk": np.ascontiguousarray(w_k[:, cols]).astype(NPDT),
            "wv": np.ascontiguousarray(w_v[:, vcols]).astype(NPDT),
            "wo": np.ascontiguousarray(w_o[vcols, :]).astype(NPDT),
            "cc": cc,
            "ss": ss,
            "masks": masks,
        })
    return in_maps


def kernel(x, w_q, w_k, w_v, w_o, _trace=False, _results_out=None):
    nc = _build()
    in_maps = _host_inputs(x, w_q, w_k, w_v, w_o)
    res = run_bass_kernel_spmd(
        nc, in_maps, core_ids=list(range(N_CORES)), trace=_trace)
    if _results_out is not None:
        _results_out.append(res)
    out = np.empty((B, S, D), np.float32)
    for b in range(B):
        acc = res.results[4 * b]["resT"].astype(np.float32)
        for g in range(1, 4):
            acc = acc + res.results[4 * b + g]["resT"]
        out[b] = acc.T
    return out


# revision 17
# speedup vs baseline: 1.0453x; 1.0453x over previous
"""Llama MHA (B=2, S=2048, D=2048, H=16, causal, RoPE) on 8 trn2 cores.

Sharding: data-parallel over batch (2 groups of 4 cores) x tensor-parallel
over heads (4 heads per core). Each core computes, for its (batch, 4 heads):
  qT/kT = w^T x^T  (features on partitions, seq on free dim)
  RoPE on qT/kT (weights column-permuted on host so even/odd feature pairs
  land de-interleaved: rows 0:64 = even, 64:128 = odd; dot products are
  permutation-invariant so scores match the reference exactly)
  scoresT[k,q] blocks -> exp (no max subtraction needed: |score*scale| <~ 6)
  -> causal mask on diagonal blocks -> PV + ones-row denominator matmuls
  -> normalize -> out projection partial resT = wo^T attnT.
Host sums the 4 partials per batch and transposes back.

All matmuls in bf16 (fp32 PSUM accumulation); softmax/normalization in fp32.
"""

import numpy as np
import ml_dtypes

import concourse.bass as bass
import concourse.mybir as mybir
import concourse.tile as tile
from concourse import bacc
from concourse.bass_utils import run_bass_kernel_spmd

B, S, D, H = 2, 2048, 2048, 16
DH = D // H            # 128 head dim
HPC = 4                # heads per core
N_CORES = 8
FH = HPC * DH          # 512 features per core
P = 128
KT = D // P            # 16 k-tiles over D
SC = S // 512          # 4 seq chunks of 512
ST = S // P            # 16 seq blocks of 128
THETA = 10000.0
SCALE = 1.0 / np.sqrt(DH)

DT = mybir.dt.bfloat16
NPDT = ml_dtypes.bfloat16

_prog_cache = {}


def _build():
    if "nc" in _prog_cache:
        return _prog_cache["nc"]
    nc = bacc.Bacc(None, target_bir_lowering=False, debug=False)

    xT = nc.dram_tensor("xT", [D, S], DT, kind="ExternalInput")
    wq = nc.dram_tensor("wq", [D, FH], DT, kind="ExternalInput")
    wk = nc.dram_tensor("wk", [D, FH], DT, kind="ExternalInput")
    wv = nc.dram_tensor("wv", [D, FH], DT, kind="ExternalInput")
    wo = nc.dram_tensor("wo", [FH, D], DT, kind="ExternalInput")
    cc = nc.dram_tensor("cc", [P, S], mybir.dt.float32, kind="ExternalInput")
    ss = nc.dram_tensor("ss", [P, S], mybir.dt.float32, kind="ExternalInput")
    masks = nc.dram_tensor("masks", [P, 4, 512], DT, kind="ExternalInput")
    resT = nc.dram_tensor("resT", [D, S], mybir.dt.float32, kind="ExternalOutput")

    f32 = mybir.dt.float32

    with tile.TileContext(nc) as tc:
        with (
            tc.tile_pool(name="persist", bufs=1) as pp,
            tc.tile_pool(name="psA", bufs=4, space="PSUM") as psA,
            tc.tile_pool(name="psO", bufs=2, space="PSUM") as psO,
            tc.tile_pool(name="psD", bufs=2, space="PSUM") as psD,
        ):
            qT = pp.tile([P, HPC, S], DT)     # per head: rows=feat, free=seq
            kT = pp.tile([P, HPC, S], DT)
            vn = pp.tile([P, ST, FH], DT)     # v natural: [seq-block, feat]
            attnT = pp.tile([P, HPC, S], DT)  # normalized attention output^T
            cc_t = pp.tile([P, S], f32)
            ss_t = pp.tile([P, S], f32)
            mask_t = pp.tile([P, 4, 512], DT)
            ones_mat = pp.tile([P, P], DT)    # denominator stationary: the
                                              # [128,128] all-ones matrix makes
                                              # every PSUM row the key-sum, so
                                              # the broadcast is free

            nc.vector.memset(ones_mat, 1.0)
            wo_t = pp.tile([P, HPC, D], DT)

            # ---------------- Phase 1: projections + RoPE -----------------
            with (
                tc.tile_pool(name="wpool", bufs=1) as wp,
                tc.tile_pool(name="xpool", bufs=2) as xp,
                tc.tile_pool(name="ropetmp", bufs=4) as rp,
            ):
                wq_t = wp.tile([P, KT, FH], DT)
                wk_t = wp.tile([P, KT, FH], DT)
                wv_t = wp.tile([P, KT, FH], DT)
                # DMA issue order is the Sync-queue order: interleave the
                # first x chunk with wq so the first matmul chain starts as
                # early as possible; defer everything not needed immediately.
                xc0 = xp.tile([P, KT, 512], DT, tag="xc", name="xc0")
                for g in range(4):
                    gs = slice(g * 4, (g + 1) * 4)
                    nc.sync.dma_start(
                        out=wq_t[:, gs, :],
                        in_=wq.rearrange("(kt p) f -> p kt f", p=P)[:, gs, :])
                    nc.sync.dma_start(
                        out=xc0[:, gs, :],
                        in_=xT.rearrange("(kt p) s -> p kt s", p=P)[:, gs, 0:512])
                nc.sync.dma_start(out=cc_t[:, 0:512], in_=cc[:, 0:512])
                nc.sync.dma_start(out=ss_t[:, 0:512], in_=ss[:, 0:512])
                for g in range(4):
                    gs = slice(g * 4, (g + 1) * 4)
                    nc.sync.dma_start(
                        out=wk_t[:, gs, :],
                        in_=wk.rearrange("(kt p) f -> p kt f", p=P)[:, gs, :])
                nc.sync.dma_start(out=wv_t, in_=wv.rearrange("(kt p) f -> p kt f", p=P))
                nc.sync.dma_start(out=mask_t, in_=masks[:, :, :])
                nc.sync.dma_start(out=cc_t[:, 512:], in_=cc[:, 512:])
                nc.sync.dma_start(out=ss_t[:, 512:], in_=ss[:, 512:])

                for sc in range(SC):
                    if sc == 0:
                        xc = xc0
                    else:
                        xc = xp.tile([P, KT, 512], DT, tag="xc", name=f"xc{sc}")
                        for g in range(4):
                            gs = slice(g * 4, (g + 1) * 4)
                            nc.sync.dma_start(
                                out=xc[:, gs, :],
                                in_=xT.rearrange("(kt p) s -> p kt s", p=P)[
                                    :, gs, sc * 512:(sc + 1) * 512],
                            )
                    if sc == 1:
                        nc.sync.dma_start(
                            out=wo_t, in_=wo.rearrange("(ft p) d -> p ft d", p=P))
                    csl = slice(sc * 512, (sc + 1) * 512)
                    # q/k projections with RoPE fused into the PSUM drain
                    for h in range(HPC):
                        fsl = slice(h * DH, (h + 1) * DH)
                        for wt, dst in ((wq_t, qT), (wk_t, kT)):
                            pq = psA.tile([P, 512], f32, tag="ps", name=f"pq{sc}{h}")
                            for k in range(KT):
                                nc.tensor.matmul(
                                    pq, wt[:, k, fsl], xc[:, k, :],
                                    start=(k == 0), stop=(k == KT - 1),
                                )
                            # RoPE: dst = pq*cc + swap(pq)*(+/-ss)
                            # ss_t rows 0:64 = +sin (feeds bottom), rows
                            # 64:128 = -sin (feeds top); swap is done by
                            # writing each product into the opposite half
                            # so every DVE op has aligned base partitions.
                            ta = rp.tile([P, 512], f32, tag="ta")
                            tb = rp.tile([P, 512], f32, tag="tb")
                            nc.vector.tensor_mul(ta, pq, cc_t[:, csl])
                            nc.vector.tensor_mul(
                                tb[0:64, :], pq[64:128, :], ss_t[64:128, csl])
                            nc.vector.tensor_mul(
                                tb[64:128, :], pq[0:64, :], ss_t[0:64, csl])
                            nc.vector.tensor_add(dst[:, h, csl], ta, tb)
                    # v projection straight into natural layout
                    for st4 in range(4):
                        sb = sc * 4 + st4
                        pv = psA.tile([P, FH], f32, tag="ps", name=f"pv{sc}{st4}")
                        for k in range(KT):
                            nc.tensor.matmul(
                                pv, xc[:, k, st4 * P:(st4 + 1) * P], wv_t[:, k, :],
                                start=(k == 0), stop=(k == KT - 1),
                            )
                        nc.vector.tensor_copy(vn[:, sb, :], pv)

            # ---------------- Phase 2: attention ------------------------
            with (
                tc.tile_pool(name="ppool", bufs=4) as ptp,
                tc.tile_pool(name="npool", bufs=4) as np_,
            ):
                for qc in range(SC):
                    qsl = slice(qc * 512, (qc + 1) * 512)
                    for h in range(HPC):
                        fsl = slice(h * DH, (h + 1) * DH)
                        po = psO.tile([P, 512], f32, tag="po", name=f"po{h}{qc}")
                        pd = psD.tile([P, 512], f32, tag="pd", name=f"pd{h}{qc}")
                        nkb = 4 * qc + 4
                        for kb in range(nkb):
                            ps = psA.tile([P, 512], f32, tag="ps",
                                          name=f"ps{h}{qc}{kb}")
                            nc.tensor.matmul(
                                ps, kT[:, h, kb * P:(kb + 1) * P], qT[:, h, qsl],
                                start=True, stop=True,
                            )
                            pt = ptp.tile([P, 512], DT, tag="pt")
                            nc.scalar.activation(
                                pt, ps, mybir.ActivationFunctionType.Exp,
                                scale=float(SCALE),
                            )
                            if kb >= 4 * qc:
                                nc.vector.tensor_mul(
                                    pt, pt, mask_t[:, kb - 4 * qc, :])
                            nc.tensor.matmul(
                                po, vn[:, kb, fsl], pt,
                                start=(kb == 0), stop=(kb == nkb - 1),
                            )
                            nc.tensor.matmul(
                                pd, ones_mat, pt,
                                start=(kb == 0), stop=(kb == nkb - 1),
                            )
                        bc = np_.tile([P, 512], f32, tag="bc")
                        nc.vector.reciprocal_approx_fast(out=bc, in_=pd)
                        nc.vector.tensor_mul(attnT[:, h, qsl], po, bc)

            # ---------------- Phase 3: output projection ----------------
            with (
                tc.tile_pool(name="rpool", bufs=4) as rop,
            ):
                for db in range(KT):
                    rt = rop.tile([P, S], f32, tag="rt")
                    for sc in range(SC):
                        csl = slice(sc * 512, (sc + 1) * 512)
                        pr = psA.tile([P, 512], f32, tag="ps", name=f"pr{sc}{db}")
                        for ft in range(HPC):
                            nc.tensor.matmul(
                                pr, wo_t[:, ft, db * P:(db + 1) * P],
                                attnT[:, ft, csl],
                                start=(ft == 0), stop=(ft == HPC - 1),
                            )
                        nc.vector.tensor_copy(rt[:, csl], pr)
                    nc.sync.dma_start(
                        out=resT[db * P:(db + 1) * P, :], in_=rt)

    nc.finalize()
    _prog_cache["nc"] = nc
    return nc


def _host_inputs(x, w_q, w_k, w_v, w_o):
    """Build the 8 per-core input maps."""
    # RoPE de-interleave permutation per head: evens then odds
    i = np.arange(DH)
    perm_head = np.concatenate([i[0::2], i[1::2]])  # within-head column order

    t = np.arange(S, dtype=np.float64)
    inv_freq = 1.0 / (THETA ** (np.arange(0, DH, 2, dtype=np.float64) / DH))
    ang = np.outer(t, inv_freq)          # [S, 64]
    cosT = np.cos(ang).T.astype(np.float32)   # [64, S]
    sinT = np.sin(ang).T.astype(np.float32)
    cc = np.vstack([cosT, cosT])         # [128, S]
    ss = np.vstack([sinT, -sinT])        # +sin feeds bottom half, -sin top

    # diagonal causal masks: mask[j][k, q] = 1 if 128*j + k <= q
    kk = np.arange(P)[:, None]
    qq = np.arange(512)[None, :]
    masks = np.stack(
        [(P * j + kk <= qq) for j in range(4)], axis=1
    ).astype(NPDT)                        # [128, 4, 512]

    in_maps = []
    for core in range(N_CORES):
        b = core // 4
        h0 = (core % 4) * HPC
        cols = np.concatenate(
            [h * DH + perm_head for h in range(h0, h0 + HPC)])   # rope-permuted
        vcols = np.arange(h0 * DH, (h0 + HPC) * DH)              # natural
        in_maps.append({
            "xT": np.ascontiguousarray(x[b].T).astype(NPDT),
            "wq": np.ascontiguousarray(w_q[:, cols]).astype(NPDT),
            "wk": np.ascontiguousarray(w_k[:, cols]).astype(NPDT),
            "wv": np.ascontiguousarray(w_v[:, vcols]).astype(NPDT),
            "wo": np.ascontiguousarray(w_o[vcols, :]).astype(NPDT),
            "cc": cc,
            "ss": ss,
            "masks": masks,
        })
    return in_maps


def kernel(x, w_q, w_k, w_v, w_o, _trace=False, _results_out=None):
    nc = _build()
    in_maps = _host_inputs(x, w_q, w_k, w_v, w_o)
    res = run_bass_kernel_spmd(
        nc, in_maps, core_ids=list(range(N_CORES)), trace=_trace)
    if _results_out is not None:
        _results_out.append(res)
    out = np.empty((B, S, D), np.float32)
    for b in range(B):
        acc = res.results[4 * b]["resT"].astype(np.float32)
        for g in range(1, 4):
            acc = acc + res.results[4 * b + g]["resT"]
        out[b] = acc.T
    return out


# revision 20
# speedup vs baseline: 1.0457x; 1.0004x over previous
"""Llama MHA (B=2, S=2048, D=2048, H=16, causal, RoPE) on 8 trn2 cores.

Sharding: data-parallel over batch (2 groups of 4 cores) x tensor-parallel
over heads (4 heads per core). Each core computes, for its (batch, 4 heads):
  qT/kT = w^T x^T  (features on partitions, seq on free dim)
  RoPE on qT/kT (weights column-permuted on host so even/odd feature pairs
  land de-interleaved: rows 0:64 = even, 64:128 = odd; dot products are
  permutation-invariant so scores match the reference exactly)
  scoresT[k,q] blocks -> exp (no max subtraction needed: |score*scale| <~ 6)
  -> causal mask on diagonal blocks -> PV + ones-row denominator matmuls
  -> normalize -> out projection partial resT = wo^T attnT.
Host sums the 4 partials per batch and transposes back.

All matmuls in bf16 (fp32 PSUM accumulation); softmax/normalization in fp32.
"""

import numpy as np
import ml_dtypes

import concourse.bass as bass
import concourse.mybir as mybir
import concourse.tile as tile
from concourse import bacc
from concourse.bass_utils import run_bass_kernel_spmd

B, S, D, H = 2, 2048, 2048, 16
DH = D // H            # 128 head dim
HPC = 4                # heads per core
N_CORES = 8
FH = HPC * DH          # 512 features per core
P = 128
KT = D // P            # 16 k-tiles over D
SC = S // 512          # 4 seq chunks of 512
ST = S // P            # 16 seq blocks of 128
THETA = 10000.0
SCALE = 1.0 / np.sqrt(DH)

DT = mybir.dt.bfloat16
NPDT = ml_dtypes.bfloat16

_prog_cache = {}


def _build():
    if "nc" in _prog_cache:
        return _prog_cache["nc"]
    nc = bacc.Bacc(None, target_bir_lowering=False, debug=False)

    xT = nc.dram_tensor("xT", [D, S], DT, kind="ExternalInput")
    wq = nc.dram_tensor("wq", [D, FH], DT, kind="ExternalInput")
    wk = nc.dram_tensor("wk", [D, FH], DT, kind="ExternalInput")
    wv = nc.dram_tensor("wv", [D, FH], DT, kind="ExternalInput")
    wo = nc.dram_tensor("wo", [FH, D], DT, kind="ExternalInput")
    cc = nc.dram_tensor("cc", [P, S], mybir.dt.float32, kind="ExternalInput")
    ss = nc.dram_tensor("ss", [P, S], mybir.dt.float32, kind="ExternalInput")
    masks = nc.dram_tensor("masks", [P, 4, 512], DT, kind="ExternalInput")
    resT = nc.dram_tensor("resT", [D, S], mybir.dt.float32, kind="ExternalOutput")

    f32 = mybir.dt.float32

    with tile.TileContext(nc) as tc:
        with (
            tc.tile_pool(name="persist", bufs=1) as pp,
            tc.tile_pool(name="psA", bufs=4, space="PSUM") as psA,
            tc.tile_pool(name="psO", bufs=2, space="PSUM") as psO,
            tc.tile_pool(name="psD", bufs=2, space="PSUM") as psD,
        ):
            qT = pp.tile([P, HPC, S], DT)     # per head: rows=feat, free=seq
            kT = pp.tile([P, HPC, S], DT)
            vn = pp.tile([P, ST, FH], DT)     # v natural: [seq-block, feat]
            attnT = pp.tile([P, HPC, S], DT)  # normalized attention output^T
            cc_t = pp.tile([P, S], f32)
            ss_t = pp.tile([P, S], f32)
            mask_t = pp.tile([P, 4, 512], DT)
            ones_mat = pp.tile([P, P], DT)    # denominator stationary: the
                                              # [128,128] all-ones matrix makes
                                              # every PSUM row the key-sum, so
                                              # the broadcast is free

            nc.vector.memset(ones_mat, 1.0)
            wo_t = pp.tile([P, HPC, D], DT)

            # ---------------- Phase 1: projections + RoPE -----------------
            with (
                tc.tile_pool(name="wpool", bufs=1) as wp,
                tc.tile_pool(name="xpool", bufs=2) as xp,
                tc.tile_pool(name="ropetmp", bufs=4) as rp,
            ):
                wq_t = wp.tile([P, KT, FH], DT)
                wk_t = wp.tile([P, KT, FH], DT)
                wv_t = wp.tile([P, KT, FH], DT)
                # DMA issue order is the Sync-queue order: interleave the
                # first x chunk with wq so the first matmul chain starts as
                # early as possible; defer everything not needed immediately.
                xc0 = xp.tile([P, KT, 512], DT, tag="xc", name="xc0")
                for g in range(4):
                    gs = slice(g * 4, (g + 1) * 4)
                    nc.sync.dma_start(
                        out=wq_t[:, gs, :],
                        in_=wq.rearrange("(kt p) f -> p kt f", p=P)[:, gs, :])
                    nc.sync.dma_start(
                        out=xc0[:, gs, :],
                        in_=xT.rearrange("(kt p) s -> p kt s", p=P)[:, gs, 0:512])
                nc.sync.dma_start(out=cc_t[:, 0:512], in_=cc[:, 0:512])
                nc.sync.dma_start(out=ss_t[:, 0:512], in_=ss[:, 0:512])
                for g in range(4):
                    gs = slice(g * 4, (g + 1) * 4)
                    nc.sync.dma_start(
                        out=wk_t[:, gs, :],
                        in_=wk.rearrange("(kt p) f -> p kt f", p=P)[:, gs, :])
                nc.sync.dma_start(out=wv_t, in_=wv.rearrange("(kt p) f -> p kt f", p=P))
                nc.sync.dma_start(out=mask_t, in_=masks[:, :, :])
                nc.sync.dma_start(out=cc_t[:, 512:], in_=cc[:, 512:])
                nc.sync.dma_start(out=ss_t[:, 512:], in_=ss[:, 512:])

                for sc in range(SC):
                    if sc == 0:
                        xc = xc0
                    else:
                        xc = xp.tile([P, KT, 512], DT, tag="xc", name=f"xc{sc}")
                        for g in range(4):
                            gs = slice(g * 4, (g + 1) * 4)
                            nc.sync.dma_start(
                                out=xc[:, gs, :],
                                in_=xT.rearrange("(kt p) s -> p kt s", p=P)[
                                    :, gs, sc * 512:(sc + 1) * 512],
                            )
                    if sc == 1:
                        nc.sync.dma_start(
                            out=wo_t, in_=wo.rearrange("(ft p) d -> p ft d", p=P))
                    csl = slice(sc * 512, (sc + 1) * 512)
                    # q/k projections with RoPE fused into the PSUM drain
                    for h in range(HPC):
                        fsl = slice(h * DH, (h + 1) * DH)
                        for wt, dst in ((wq_t, qT), (wk_t, kT)):
                            pq = psA.tile([P, 512], f32, tag="ps", name=f"pq{sc}{h}")
                            for k in range(KT):
                                nc.tensor.matmul(
                                    pq, wt[:, k, fsl], xc[:, k, :],
                                    start=(k == 0), stop=(k == KT - 1),
                                )
                            # RoPE: dst = pq*cc + swap(pq)*(+/-ss)
                            # ss_t rows 0:64 = +sin (feeds bottom), rows
                            # 64:128 = -sin (feeds top); swap is done by
                            # writing each product into the opposite half
                            # so every DVE op has aligned base partitions.
                            ta = rp.tile([P, 512], f32, tag="ta")
                            tb = rp.tile([P, 512], f32, tag="tb")
                            nc.vector.tensor_mul(ta, pq, cc_t[:, csl])
                            nc.vector.tensor_mul(
                                tb[0:64, :], pq[64:128, :], ss_t[64:128, csl])
                            nc.vector.tensor_mul(
                                tb[64:128, :], pq[0:64, :], ss_t[0:64, csl])
                            nc.vector.tensor_add(dst[:, h, csl], ta, tb)
                    # v projection straight into natural layout
                    for st4 in range(4):
                        sb = sc * 4 + st4
                        pv = psA.tile([P, FH], f32, tag="ps", name=f"pv{sc}{st4}")
                        for k in range(KT):
                            nc.tensor.matmul(
                                pv, xc[:, k, st4 * P:(st4 + 1) * P], wv_t[:, k, :],
                                start=(k == 0), stop=(k == KT - 1),
                            )
                        nc.vector.tensor_copy(vn[:, sb, :], pv)

            # ---------------- Phase 2: attention ------------------------
            with (
                tc.tile_pool(name="ppool", bufs=6) as ptp,
                tc.tile_pool(name="npool", bufs=4) as np_,
            ):
                for qc in range(SC):
                    qsl = slice(qc * 512, (qc + 1) * 512)
                    for h in range(HPC):
                        fsl = slice(h * DH, (h + 1) * DH)
                        po = psO.tile([P, 512], f32, tag="po", name=f"po{h}{qc}")
                        pd = psD.tile([P, 512], f32, tag="pd", name=f"pd{h}{qc}")
                        nkb = 4 * qc + 4
                        for kb in range(nkb):
                            ps = psA.tile([P, 512], f32, tag="ps",
                                          name=f"ps{h}{qc}{kb}")
                            nc.tensor.matmul(
                                ps, kT[:, h, kb * P:(kb + 1) * P], qT[:, h, qsl],
                                start=True, stop=True,
                            )
                            pt = ptp.tile([P, 512], DT, tag="pt")
                            nc.scalar.activation(
                                pt, ps, mybir.ActivationFunctionType.Exp,
                                scale=float(SCALE),
                            )
                            if kb >= 4 * qc:
                                nc.vector.tensor_mul(
                                    pt, pt, mask_t[:, kb - 4 * qc, :])
                            nc.tensor.matmul(
                                po, vn[:, kb, fsl], pt,
                                start=(kb == 0), stop=(kb == nkb - 1),
                            )
                            nc.tensor.matmul(
                                pd, ones_mat, pt,
                                start=(kb == 0), stop=(kb == nkb - 1),
                            )
                        bc = np_.tile([P, 512], f32, tag="bc")
                        nc.vector.reciprocal_approx_fast(out=bc, in_=pd)
                        nc.vector.tensor_mul(attnT[:, h, qsl], po, bc)

            # ---------------- Phase 3: output projection ----------------
            with (
                tc.tile_pool(name="rpool", bufs=4) as rop,
            ):
                for db in range(KT):
                    rt = rop.tile([P, S], f32, tag="rt")
                    for sc in range(SC):
                        csl = slice(sc * 512, (sc + 1) * 512)
                        pr = psA.tile([P, 512], f32, tag="ps", name=f"pr{sc}{db}")
                        for ft in range(HPC):
                            nc.tensor.matmul(
                                pr, wo_t[:, ft, db * P:(db + 1) * P],
                                attnT[:, ft, csl],
                                start=(ft == 0), stop=(ft == HPC - 1),
                            )
                        nc.vector.tensor_copy(rt[:, csl], pr)
                    nc.sync.dma_start(
                        out=resT[db * P:(db + 1) * P, :], in_=rt)

    nc.finalize()
    _prog_cache["nc"] = nc
    return nc


def _host_inputs(x, w_q, w_k, w_v, w_o):
    """Build the 8 per-core input maps."""
    # RoPE de-interleave permutation per head: evens then odds
    i = np.arange(DH)
    perm_head = np.concatenate([i[0::2], i[1::2]])  # within-head column order

    t = np.arange(S, dtype=np.float64)
    inv_freq = 1.0 / (THETA ** (np.arange(0, DH, 2, dtype=np.float64) / DH))
    ang = np.outer(t, inv_freq)          # [S, 64]
    cosT = np.cos(ang).T.astype(np.float32)   # [64, S]
    sinT = np.sin(ang).T.astype(np.float32)
    cc = np.vstack([cosT, cosT])         # [128, S]
    ss = np.vstack([sinT, -sinT])        # +sin feeds bottom half, -sin top

    # diagonal causal masks: mask[j][k, q] = 1 if 128*j + k <= q
    kk = np.arange(P)[:, None]
    qq = np.arange(512)[None, :]
    masks = np.stack(
        [(P * j + kk <= qq) for j in range(4)], axis=1
    ).astype(NPDT)                        # [128, 4, 512]

    in_maps = []
    for core in range(N_CORES):
        b = core // 4
        h0 = (core % 4) * HPC
        cols = np.concatenate(
            [h * DH + perm_head for h in range(h0, h0 + HPC)])   # rope-permuted
        vcols = np.arange(h0 * DH, (h0 + HPC) * DH)              # natural
        in_maps.append({
            "xT": np.ascontiguousarray(x[b].T).astype(NPDT),
            "wq": np.ascontiguousarray(w_q[:, cols]).astype(NPDT),
            "wk": np.ascontiguousarray(w_k[:, cols]).astype(NPDT),
            "wv": np.ascontiguousarray(w_v[:, vcols]).astype(NPDT),
            "wo": np.ascontiguousarray(w_o[vcols, :]).astype(NPDT),
            "cc": cc,
            "ss": ss,
            "masks": masks,
        })
    return in_maps


def kernel(x, w_q, w_k, w_v, w_o, _trace=False, _results_out=None):
    x = np.asarray(x, dtype=np.float32)
    w_q = np.asarray(w_q, dtype=np.float32)
    w_k = np.asarray(w_k, dtype=np.float32)
    w_v = np.asarray(w_v, dtype=np.float32)
    w_o = np.asarray(w_o, dtype=np.float32)
    nc = _build()
    in_maps = _host_inputs(x, w_q, w_k, w_v, w_o)
    res = run_bass_kernel_spmd(
        nc, in_maps, core_ids=list(range(N_CORES)), trace=_trace)
    if _results_out is not None:
        _results_out.append(res)
    out = np.empty((B, S, D), np.float32)
    for b in range(B):
        acc = res.results[4 * b]["resT"].astype(np.float32)
        for g in range(1, 4):
            acc = acc + res.results[4 * b + g]["resT"]
        out[b] = acc.T
    return out
